# Initial kernel scaffold
#
"""Trainium2 Bass kernel for StyleGAN2-style 4x4 blur (upfirdn2d, up=down=1,
pad=(2,1)) on x:[8,128,256,256] fp32.

Math: out[i,j] = sum_{p,q in [-2,1]} K[1-p,1-q] * x[i+p, j+q]  (zero-padded),
with K the 4x4 blur kernel. K is rank-1 (outer product), so the conv is
separable: an H-pass with taps from the column factor and a W-pass with taps
from the row factor.

Mapping to hardware: each 1-D conv is a banded-matrix product. Per (b,c)
image (256x256) we run two PSUM-accumulated matmul groups on TensorE using
float32r (relaxed fp32, full-rate at N>=256):

  MM1:  t1[w, h'] = sum_h x[h, w] * BH[h, h']      (H-conv, output transposed)
  MM2:  y[h', w'] = sum_w t1[w, h'] * BW[w, w']    (W-conv, transposes back)

K (contraction) is capped at 128, so each group is 2 accumulating matmuls
over 128-row halves; the 256-wide bands fold the zero padding at the image
borders. ScalarE/VectorE evacuate PSUM->SBUF; HWDGE DMAs stream HBM.

Sharding: batch dim (8) -> one NeuronCore each; channels (128) map to
sequential images per core.
"""

import sys

sys.path.insert(0, "/opt/trn_rl_repo")

import numpy as np

B, C, H, W = 8, 128, 256, 256
KH = KW = 4
N_CORES = 8


def _band_256(taps):
    """Band matrix Bd[k, n] = taps[1 + n - k] for 0 <= 1+n-k < 4, else 0.

    t_out[n] = sum_k Bd[k, n] * x_in[k] is the 1-D conv
    out[n] = sum_{p=-2..1} taps_coeff[p] x[n+p] with taps_coeff[p] = taps[1-p]
    and zero padding (2 leading, 1 trailing) folded in by truncation.
    """
    Bd = np.zeros((256, 256), dtype=np.float64)
    for n in range(256):
        for d in range(4):
            k = n + 1 - d
            if 0 <= k < 256:
                Bd[k, n] = taps[d]
    return Bd


def _factor_kernel(k2):
    """Rank-1 factorization k2 = outer(u, v) (k2 is an outer product)."""
    k2 = np.asarray(k2, dtype=np.float64)
    uu, ss, vv = np.linalg.svd(k2)
    assert ss[1] < 1e-5 * max(ss[0], 1e-30), "blur kernel is not rank-1"
    u = uu[:, 0] * np.sqrt(ss[0])
    v = vv[0] * np.sqrt(ss[0])
    # fix sign so that outer(u, v) ~ k2 with u mostly positive
    if u.sum() < 0:
        u, v = -u, -v
    return u, v


def _make_bands(k2):
    """Returns (bh_sb, bw_sb) as float32 [128, 512] SBUF layouts.

    bh_sb[p, kh*256 + n] = BH[kh*128 + p, n];  same for bw_sb.
    """
    u, v = _factor_kernel(k2)
    # coefficient of x[i+p] is u[1-p] -> band entry BH[k, n] = u[1 + n - k]
    BH = _band_256(u)
    BW = _band_256(v)

    def to_sb(Bd):
        return (
            Bd.reshape(2, 128, 256).transpose(1, 0, 2).reshape(128, 512)
        ).astype(np.float32)

    return to_sb(BH), to_sb(BW)


_NC_CACHE = {}


def _build_nc(n_images):
    import concourse.bacc as bacc
    import concourse.mybir as mybir
    from concourse.tile import TileContext

    f32 = mybir.dt.float32
    f32r = mybir.dt.float32r

    nc = bacc.Bacc("TRN2", target_bir_lowering=False)
    x = nc.dram_tensor("x", (n_images, 256, 256), f32, kind="ExternalInput")
    bh = nc.dram_tensor("bh", (128, 512), f32, kind="ExternalInput")
    bw = nc.dram_tensor("bw", (128, 512), f32, kind="ExternalInput")
    y = nc.dram_tensor("y", (n_images, 256, 256), f32, kind="ExternalOutput")

    # DRAM view: image c, partition p=row-within-half, free = (half, col)
    x_v = x.rearrange("c (kh p) w -> c p kh w", p=128)
    y_v = y.rearrange("c (hb p) w -> c p hb w", p=128)

    with TileContext(nc) as tc:
        with (
            tc.tile_pool(name="consts", bufs=1) as cpool,
            tc.tile_pool(name="xt", bufs=6) as xpool,
            tc.tile_pool(name="t1", bufs=3) as tpool,
            tc.tile_pool(name="yt", bufs=4) as ypool,
            tc.tile_pool(name="ps1", bufs=2, space="PSUM") as ps1pool,
            tc.tile_pool(name="ps2", bufs=2, space="PSUM") as ps2pool,
        ):
            bh_sb = cpool.tile([128, 512], f32, tag="bh")
            bw_sb = cpool.tile([128, 512], f32, tag="bw")
            nc.sync.dma_start(out=bh_sb[:], in_=bh[:])
            nc.sync.dma_start(out=bw_sb[:], in_=bw[:])

            for c in range(n_images):
                xt = xpool.tile([128, 512], f32)
                nc.sync.dma_start(
                    out=xt[:].rearrange("p (kh w) -> p kh w", kh=2),
                    in_=x_v[c],
                )

                # MM1: t1[w, h'] = sum_h x[h, w] * BH[h, h']
                ps1 = ps1pool.tile([128, 512], f32)
                for wb in range(2):
                    for kh in range(2):
                        lhsT = xt[:, kh * 256 + wb * 128 : kh * 256 + wb * 128 + 128]
                        rhs = bh_sb[:, kh * 256 : (kh + 1) * 256]
                        nc.tensor.matmul(
                            ps1[:, wb * 256 : (wb + 1) * 256],
                            lhsT.bitcast(f32r),
                            rhs.bitcast(f32r),
                            start=(kh == 0),
                            stop=(kh == 1),
                        )

                t1 = tpool.tile([128, 512], f32)
                nc.scalar.copy(out=t1[:], in_=ps1[:])

                # MM2: y[h', w'] = sum_w t1[w, h'] * BW[w, w']
                ps2 = ps2pool.tile([128, 512], f32)
                for hb in range(2):
                    for wb in range(2):
                        lhsT = t1[:, wb * 256 + hb * 128 : wb * 256 + hb * 128 + 128]
                        rhs = bw_sb[:, wb * 256 : (wb + 1) * 256]
                        nc.tensor.matmul(
                            ps2[:, hb * 256 : (hb + 1) * 256],
                            lhsT.bitcast(f32r),
                            rhs.bitcast(f32r),
                            start=(wb == 0),
                            stop=(wb == 1),
                        )

                yt = ypool.tile([128, 512], f32)
                nc.vector.tensor_copy(out=yt[:], in_=ps2[:])
                nc.scalar.dma_start(
                    out=y_v[c],
                    in_=yt[:].rearrange("p (hb w) -> p hb w", hb=2),
                )

    nc.compile()
    return nc


def _get_nc(n_images):
    if n_images not in _NC_CACHE:
        _NC_CACHE[n_images] = _build_nc(n_images)
    return _NC_CACHE[n_images]


def kernel(x, kernel, _trace=False):
    from concourse import bass_utils

    x = np.ascontiguousarray(np.asarray(x), dtype=np.float32)
    k2 = np.asarray(kernel, dtype=np.float32)
    assert x.shape == (B, C, H, W), x.shape
    assert k2.shape == (KH, KW), k2.shape

    bh_sb, bw_sb = _make_bands(k2)

    nc = _get_nc(C)
    in_maps = [{"x": x[b], "bh": bh_sb, "bw": bw_sb} for b in range(B)]
    res = bass_utils.run_bass_kernel_spmd(
        nc, in_maps, core_ids=list(range(N_CORES)), trace=_trace
    )
    out = np.stack([res.results[b]["y"] for b in range(B)], axis=0)
    if _trace:
        return out, res
    return out


# revision 9
# speedup vs baseline: 1.1060x; 1.1060x over previous
"""Trainium2 Bass kernel for StyleGAN2-style 4x4 blur (upfirdn2d, up=down=1,
pad=(2,1)) on x:[8,128,256,256] fp32.

Math: out[i,j] = sum_{p,q in [-2,1]} K[1-p,1-q] * x[i+p, j+q]  (zero-padded),
with K the 4x4 blur kernel. K is rank-1 (outer product), so the conv is
separable: an H-pass with taps from the column factor and a W-pass with taps
from the row factor.

Mapping to hardware: each 1-D conv is a banded-matrix product. Per (b,c)
image (256x256) we run two PSUM-accumulated matmul groups on TensorE using
float32r (relaxed fp32, full-rate at N>=256):

  MM1:  t1[w, h'] = sum_h x[h, w] * BH[h, h']      (H-conv, output transposed)
  MM2:  y[h', w'] = sum_w t1[w, h'] * BW[w, w']    (W-conv, transposes back)

K (contraction) is capped at 128, so each group is 2 accumulating matmuls
over 128-row halves; the 256-wide bands fold the zero padding at the image
borders. ScalarE/VectorE evacuate PSUM->SBUF; HWDGE DMAs stream HBM.

Sharding: batch dim (8) -> one NeuronCore each; channels (128) map to
sequential images per core.
"""

import sys

sys.path.insert(0, "/opt/trn_rl_repo")

import numpy as np

B, C, H, W = 8, 128, 256, 256
KH = KW = 4
N_CORES = 8


def _band_256(taps):
    """Band matrix Bd[k, n] = taps[1 + n - k] for 0 <= 1+n-k < 4, else 0.

    t_out[n] = sum_k Bd[k, n] * x_in[k] is the 1-D conv
    out[n] = sum_{p=-2..1} taps_coeff[p] x[n+p] with taps_coeff[p] = taps[1-p]
    and zero padding (2 leading, 1 trailing) folded in by truncation.
    """
    Bd = np.zeros((256, 256), dtype=np.float64)
    for n in range(256):
        for d in range(4):
            k = n + 1 - d
            if 0 <= k < 256:
                Bd[k, n] = taps[d]
    return Bd


def _factor_kernel(k2):
    """Rank-1 factorization k2 = outer(u, v) (k2 is an outer product)."""
    k2 = np.asarray(k2, dtype=np.float64)
    uu, ss, vv = np.linalg.svd(k2)
    assert ss[1] < 1e-5 * max(ss[0], 1e-30), "blur kernel is not rank-1"
    u = uu[:, 0] * np.sqrt(ss[0])
    v = vv[0] * np.sqrt(ss[0])
    # fix sign so that outer(u, v) ~ k2 with u mostly positive
    if u.sum() < 0:
        u, v = -u, -v
    return u, v


def _make_bands(k2):
    """Returns (bh_sb, bw_sb) as float32 [128, 512] SBUF layouts.

    bh_sb[p, kh*256 + n] = BH[kh*128 + p, n];  same for bw_sb.
    """
    u, v = _factor_kernel(k2)
    # coefficient of x[i+p] is u[1-p] -> band entry BH[k, n] = u[1 + n - k]
    BH = _band_256(u)
    BW = _band_256(v)

    def to_sb(Bd):
        return (
            Bd.reshape(2, 128, 256).transpose(1, 0, 2).reshape(128, 512)
        ).astype(np.float32)

    return to_sb(BH), to_sb(BW)


_NC_CACHE = {}


def _build_nc(n_images, repeats=1):
    import concourse.bacc as bacc
    import concourse.mybir as mybir
    from concourse.tile import TileContext

    f32 = mybir.dt.float32
    f32r = mybir.dt.float32r

    nc = bacc.Bacc("TRN2", target_bir_lowering=False)
    x = nc.dram_tensor("x", (n_images, 256, 256), f32r, kind="ExternalInput")
    bh = nc.dram_tensor("bh", (128, 512), f32r, kind="ExternalInput")
    bw = nc.dram_tensor("bw", (128, 512), f32r, kind="ExternalInput")
    y = nc.dram_tensor("y", (n_images, 256, 256), f32, kind="ExternalOutput")

    # DRAM view: image c, partition p=row-within-half, free = (half, col)
    x_v = x.rearrange("c (kh p) w -> c p kh w", p=128)
    y_v = y.rearrange("c (hb p) w -> c p hb w", p=128)

    with TileContext(nc) as tc:
        with (
            tc.tile_pool(name="consts", bufs=1) as cpool,
            tc.tile_pool(name="xt", bufs=6) as xpool,
            tc.tile_pool(name="t1", bufs=3) as tpool,
            tc.tile_pool(name="yt", bufs=4) as ypool,
            tc.tile_pool(name="ps1", bufs=2, space="PSUM") as ps1pool,
            tc.tile_pool(name="ps2", bufs=2, space="PSUM") as ps2pool,
        ):
            bh_sb = cpool.tile([128, 512], f32r, tag="bh")
            bw_sb = cpool.tile([128, 512], f32r, tag="bw")
            nc.sync.dma_start(out=bh_sb[:], in_=bh[:])
            nc.sync.dma_start(out=bw_sb[:], in_=bw[:])

            for c in [i for _ in range(repeats) for i in range(n_images)]:
                xt = xpool.tile([128, 512], f32r)
                nc.sync.dma_start(
                    out=xt[:].rearrange("p (kh w) -> p kh w", kh=2),
                    in_=x_v[c],
                )

                # MM1: t1[w, h'] = sum_h x[h, w] * BH[h, h']
                ps1 = ps1pool.tile([128, 512], f32)
                for wb in range(2):
                    for kh in range(2):
                        lhsT = xt[:, kh * 256 + wb * 128 : kh * 256 + wb * 128 + 128]
                        rhs = bh_sb[:, kh * 256 : (kh + 1) * 256]
                        nc.tensor.matmul(
                            ps1[:, wb * 256 : (wb + 1) * 256],
                            lhsT,
                            rhs,
                            start=(kh == 0),
                            stop=(kh == 1),
                        )

                t1 = tpool.tile([128, 512], f32r)
                nc.scalar.copy(out=t1[:], in_=ps1[:])

                # MM2: y[h', w'] = sum_w t1[w, h'] * BW[w, w']
                ps2 = ps2pool.tile([128, 512], f32)
                for hb in range(2):
                    for wb in range(2):
                        lhsT = t1[:, wb * 256 + hb * 128 : wb * 256 + hb * 128 + 128]
                        rhs = bw_sb[:, wb * 256 : (wb + 1) * 256]
                        nc.tensor.matmul(
                            ps2[:, hb * 256 : (hb + 1) * 256],
                            lhsT,
                            rhs,
                            start=(wb == 0),
                            stop=(wb == 1),
                        )

                yt = ypool.tile([128, 512], f32)
                nc.vector.tensor_copy(out=yt[:], in_=ps2[:])
                nc.scalar.dma_start(
                    out=y_v[c],
                    in_=yt[:].rearrange("p (hb w) -> p hb w", hb=2),
                )

    nc.compile()
    return nc


def _get_nc(n_images, repeats=1):
    key = (n_images, repeats)
    if key not in _NC_CACHE:
        _NC_CACHE[key] = _build_nc(n_images, repeats)
    return _NC_CACHE[key]


def kernel(x, kernel, _trace=False):
    from concourse import bass_utils

    x = np.ascontiguousarray(np.asarray(x), dtype=np.float32)
    k2 = np.asarray(kernel, dtype=np.float32)
    assert x.shape == (B, C, H, W), x.shape
    assert k2.shape == (KH, KW), k2.shape

    bh_sb, bw_sb = _make_bands(k2)

    nc = _get_nc(C)
    in_maps = [{"x": x[b], "bh": bh_sb, "bw": bw_sb} for b in range(B)]
    res = bass_utils.run_bass_kernel_spmd(
        nc, in_maps, core_ids=list(range(N_CORES)), trace=_trace
    )
    out = np.stack([res.results[b]["y"] for b in range(B)], axis=0)
    if _trace:
        return out, res
    return out


# revision 13
# speedup vs baseline: 19.8750x; 17.9705x over previous
"""Trainium2 Bass kernel for StyleGAN2-style 4x4 blur (upfirdn2d, up=down=1,
pad=(2,1)) on x:[8,128,256,256] fp32.

Math: out[i,j] = sum_{p,q in [-2,1]} K[1-p,1-q] * x[i+p, j+q]  (zero-padded),
with K the 4x4 blur kernel. K is rank-1 (outer product), so the conv is
separable: an H-pass with taps from the column factor and a W-pass with taps
from the row factor.

Mapping to hardware: each 1-D conv is a banded-matrix product. Per (b,c)
image (256x256) we run two PSUM-accumulated matmul groups on TensorE using
float32r (relaxed fp32, full-rate at N>=256):

  MM1:  t1[w, h'] = sum_h x[h, w] * BH[h, h']      (H-conv, output transposed)
  MM2:  y[h', w'] = sum_w t1[w, h'] * BW[w, w']    (W-conv, transposes back)

K (contraction) is capped at 128, so each group is 2 accumulating matmuls
over 128-row halves; the 256-wide bands fold the zero padding at the image
borders. ScalarE/VectorE evacuate PSUM->SBUF; HWDGE DMAs stream HBM.

Sharding: batch dim (8) -> one NeuronCore each; channels (128) map to
sequential images per core.
"""

import sys

sys.path.insert(0, "/opt/trn_rl_repo")

import numpy as np

B, C, H, W = 8, 128, 256, 256
KH = KW = 4
N_CORES = 8


def _band_256(taps):
    """Band matrix Bd[k, n] = taps[1 + n - k] for 0 <= 1+n-k < 4, else 0.

    t_out[n] = sum_k Bd[k, n] * x_in[k] is the 1-D conv
    out[n] = sum_{p=-2..1} taps_coeff[p] x[n+p] with taps_coeff[p] = taps[1-p]
    and zero padding (2 leading, 1 trailing) folded in by truncation.
    """
    Bd = np.zeros((256, 256), dtype=np.float64)
    for n in range(256):
        for d in range(4):
            k = n + 1 - d
            if 0 <= k < 256:
                Bd[k, n] = taps[d]
    return Bd


def _factor_kernel(k2):
    """Rank-1 factorization k2 = outer(u, v) (k2 is an outer product)."""
    k2 = np.asarray(k2, dtype=np.float64)
    uu, ss, vv = np.linalg.svd(k2)
    assert ss[1] < 1e-5 * max(ss[0], 1e-30), "blur kernel is not rank-1"
    u = uu[:, 0] * np.sqrt(ss[0])
    v = vv[0] * np.sqrt(ss[0])
    # fix sign so that outer(u, v) ~ k2 with u mostly positive
    if u.sum() < 0:
        u, v = -u, -v
    return u, v


def _make_bands(k2):
    """Returns (bh_sb, bw_sb) as float32 [128, 512] SBUF layouts.

    bh_sb[p, kh*256 + n] = BH[kh*128 + p, n];  same for bw_sb.
    """
    u, v = _factor_kernel(k2)
    # coefficient of x[i+p] is u[1-p] -> band entry BH[k, n] = u[1 + n - k]
    BH = _band_256(u)
    BW = _band_256(v)

    def to_sb(Bd):
        return (
            Bd.reshape(2, 128, 256).transpose(1, 0, 2).reshape(128, 512)
        ).astype(np.float32)

    return to_sb(BH), to_sb(BW)


_NC_CACHE = {}


def _build_nc(n_images, repeats=1, mode="full"):
    import concourse.bacc as bacc
    import concourse.mybir as mybir
    from concourse.tile import TileContext

    f32 = mybir.dt.float32
    f32r = mybir.dt.float32r

    nc = bacc.Bacc("TRN2", target_bir_lowering=False)
    x = nc.dram_tensor("x", (n_images, 256, 256), f32r, kind="ExternalInput")
    bh = nc.dram_tensor("bh", (128, 512), f32r, kind="ExternalInput")
    bw = nc.dram_tensor("bw", (128, 512), f32r, kind="ExternalInput")
    y = nc.dram_tensor("y", (n_images, 256, 256), f32, kind="ExternalOutput")

    # DRAM view: image c, partition p=row-within-half, free = (half, col)
    x_v = x.rearrange("c (kh p) w -> c p kh w", p=128)
    y_v = y.rearrange("c (hb p) w -> c p hb w", p=128)

    with TileContext(nc) as tc:
        with (
            tc.tile_pool(name="consts", bufs=1) as cpool,
            tc.tile_pool(name="xt", bufs=6) as xpool,
            tc.tile_pool(name="t1", bufs=3) as tpool,
            tc.tile_pool(name="yt", bufs=4) as ypool,
            tc.tile_pool(name="ps1", bufs=2, space="PSUM") as ps1pool,
            tc.tile_pool(name="ps2", bufs=2, space="PSUM") as ps2pool,
        ):
            bh_sb = cpool.tile([128, 512], f32r, tag="bh")
            bw_sb = cpool.tile([128, 512], f32r, tag="bw")
            nc.sync.dma_start(out=bh_sb[:], in_=bh[:])
            nc.sync.dma_start(out=bw_sb[:], in_=bw[:])

            for c in [i for _ in range(repeats) for i in range(n_images)]:
                xt = xpool.tile([128, 512], f32r)
                nc.sync.dma_start(
                    out=xt[:].rearrange("p (kh w) -> p kh w", kh=2),
                    in_=x_v[c],
                )
                if mode == "dmaonly":
                    nc.scalar.dma_start(
                        out=y_v[c],
                        in_=xt[:].bitcast(f32).rearrange("p (hb w) -> p hb w", hb=2),
                    )
                    continue

                # MM1: t1[w, h'] = sum_h x[h, w] * BH[h, h']
                ps1 = ps1pool.tile([128, 512], f32)
                for wb in range(2):
                    for kh in range(2):
                        lhsT = xt[:, kh * 256 + wb * 128 : kh * 256 + wb * 128 + 128]
                        rhs = bh_sb[:, kh * 256 : (kh + 1) * 256]
                        nc.tensor.matmul(
                            ps1[:, wb * 256 : (wb + 1) * 256],
                            lhsT,
                            rhs,
                            start=(kh == 0),
                            stop=(kh == 1),
                        )

                t1 = tpool.tile([128, 512], f32r)
                nc.scalar.copy(out=t1[:], in_=ps1[:])

                # MM2: y[h', w'] = sum_w t1[w, h'] * BW[w, w']
                ps2 = ps2pool.tile([128, 512], f32)
                for hb in range(2):
                    for wb in range(2):
                        lhsT = t1[:, wb * 256 + hb * 128 : wb * 256 + hb * 128 + 128]
                        rhs = bw_sb[:, wb * 256 : (wb + 1) * 256]
                        nc.tensor.matmul(
                            ps2[:, hb * 256 : (hb + 1) * 256],
                            lhsT,
                            rhs,
                            start=(wb == 0),
                            stop=(wb == 1),
                        )

                yt = ypool.tile([128, 512], f32)
                nc.vector.tensor_copy(out=yt[:], in_=ps2[:])
                nc.scalar.dma_start(
                    out=y_v[c],
                    in_=yt[:].rearrange("p (hb w) -> p hb w", hb=2),
                )

    nc.compile()
    return nc


def _get_nc(n_images, repeats=1, mode="full"):
    key = (n_images, repeats, mode)
    if key not in _NC_CACHE:
        _NC_CACHE[key] = _build_nc(n_images, repeats, mode)
    return _NC_CACHE[key]


def kernel(x, kernel, _trace=False):
    from concourse import bass_utils

    x = np.ascontiguousarray(np.asarray(x), dtype=np.float32)
    k2 = np.asarray(kernel, dtype=np.float32)
    assert x.shape == (B, C, H, W), x.shape
    assert k2.shape == (KH, KW), k2.shape

    bh_sb, bw_sb = _make_bands(k2)

    nc = _get_nc(C)
    in_maps = [{"x": x[b], "bh": bh_sb, "bw": bw_sb} for b in range(B)]
    res = bass_utils.run_bass_kernel_spmd(
        nc, in_maps, core_ids=list(range(N_CORES)), trace=_trace
    )
    out = np.stack([res.results[b]["y"] for b in range(B)], axis=0)
    if _trace:
        return out, res
    return out
